# revision 14
# baseline (speedup 1.0000x reference)
"""Trainium2 Bass kernel for MetapathAggreLayer.

Computes, per node n:
    score[n, c] = sum_h hs[c, n, h] * v[c, h]        (c = 8 channels)
    att = softmax(score, axis=c)
    out[n, h]   = sum_c att[n, c] * hs[c, n, h]

Sharding: data-parallel over nodes across 8 NeuronCores (50000 nodes each).

hs is staged twice in fp16 (same total bytes as fp32 once): an [n, c, h]
interleave for the attention-weighted sum, and a [c, h, n] transpose so the
score dot-products run on the TensorEngine (contraction over h = partitions).
Output returns fp16, cast to fp32 on the host.

Per 512-node macro-tile:
  PE:  score via 8 matmuls (v_c column x hsT panel -> one PSUM row each),
       then 4 transposes to bring score back to node-partitions
  ACT: exp, part of the per-channel att*hs scalings, score PSUM->SBUF copy
  DVE: softmax sums/reciprocal/normalize, the other att*hs scalings, and the
       channel sum as a 3-level fp16 tree-add
  Pool: SWDGE descriptor-gen for the transposed loads (otherwise idle)
"""

import sys

if "/opt/trn_rl_repo" not in sys.path:
    sys.path.insert(0, "/opt/trn_rl_repo")

import numpy as np

NCH = 8
NNODE = 400000
NHID = 128
NCORES = 8
NPC = NNODE // NCORES  # 50000
P = 128
NG = 4
FULL_ITERS = NPC // (NG * P)  # 97
TAIL = NPC - FULL_ITERS * NG * P  # 336 = 3 * 112

N_ACT_WSUM = 11  # of 32 att*hs scalings, how many go to ACT (rest DVE)

_cache = {}


def _build_program():
    import concourse.bacc as bacc
    import concourse.tile as tile
    import concourse.mybir as mybir

    dt = mybir.dt
    AX = mybir.AxisListType
    AF = mybir.ActivationFunctionType

    nc = bacc.Bacc("TRN2", target_bir_lowering=False, debug=False)
    hs_d = nc.dram_tensor("hs", [NPC, NCH, NHID], dt.float16, kind="ExternalInput").ap()
    hsT_d = nc.dram_tensor(
        "hsT", [NCH, NHID, NPC], dt.float16, kind="ExternalInput"
    ).ap()
    vc_d = nc.dram_tensor("vpad", [NHID, NCH, NCH], dt.float16, kind="ExternalInput").ap()
    i32_d = nc.dram_tensor("ident8", [8, 8], dt.float32, kind="ExternalInput").ap()
    out_d = nc.dram_tensor("out", [NPC, NHID], dt.float16, kind="ExternalOutput").ap()

    with tile.TileContext(nc) as tc:
        with (
            tc.tile_pool(name="const", bufs=1) as cpool,
            tc.tile_pool(name="hs", bufs=3) as hpool,
            tc.tile_pool(name="hsT", bufs=3) as thpool,
            tc.tile_pool(name="wsum", bufs=2) as wpool,
            tc.tile_pool(name="csum", bufs=2) as cspool,
            tc.tile_pool(name="small", bufs=6) as spool,
            tc.tile_pool(name="outp", bufs=4) as opool,
            tc.tile_pool(name="ps", bufs=2, space="PSUM") as pspool,
        ):
            vpad = cpool.tile([NHID, NCH, NCH], dt.float16)
            nc.sync.dma_start(vpad[:], vc_d[:])
            I8 = cpool.tile([8, 8], dt.float32)
            nc.sync.dma_start(I8[:], i32_d[:])
            bconst = cpool.tile([P, 1], dt.float32)
            nc.vector.memset(bconst[:], -5.545177444479562)

            def body(base, ng, p):
                n = ng * p
                hs_t = hpool.tile([P, NG, NCH, NHID], dt.float16, tag="hs")
                for g in range(ng):
                    eng = nc.scalar if g == 3 else nc.sync
                    eng.dma_start(
                        hs_t[:p, g],
                        hs_d[base + g * p : base + (g + 1) * p],
                    )
                hsT_t = thpool.tile([NHID, NCH, NG * P], dt.float16, tag="hsT")
                for q in range(4):
                    deng = nc.sync if q == 3 else nc.gpsimd
                    deng.dma_start(
                        hsT_t[:, 2 * q : 2 * q + 2, 0:n],
                        hsT_d[2 * q : 2 * q + 2, :, base : base + n].rearrange(
                            "c h n -> h c n"
                        ),
                    )

                # score on PE: row c of scT <- v_c . hsT_c
                scT = pspool.tile([NCH, NG * P], dt.float32, tag="scT")
                for c in range(NCH):
                    nc.tensor.matmul(
                        scT[0:NCH, 0:n],
                        vpad[:, c, :],
                        hsT_t[:, c, 0:n],
                        start=(c == 0),
                        stop=(c == NCH - 1),
                    )
                # exp directly on the transposed score (biased; bias cancels
                # in r), then transpose e back to node partitions
                eT = spool.tile([NCH, NG * P], dt.float32, tag="eT")
                nc.scalar.activation(
                    eT[:, 0:n], scT[:, 0:n], AF.Exp, bias=bconst[0:NCH]
                )
                eN = pspool.tile([P, NG, NCH], dt.float32, tag="eN")
                for g in range(ng):
                    nc.tensor.transpose(
                        eN[:p, g], eT[:, g * p : (g + 1) * p], I8[:]
                    )
                s32 = spool.tile([P, NG], dt.float32, tag="s32")
                nc.vector.reduce_sum(s32[:p, 0:ng], eN[:p, 0:ng], axis=AX.X)
                r32 = spool.tile([P, NG], dt.float32, tag="r32")
                nc.vector.reciprocal(r32[:p, 0:ng], s32[:p, 0:ng])
                att32 = spool.tile([P, NG, NCH], dt.float32, tag="att32")
                for g in range(ng):
                    nc.vector.tensor_scalar_mul(
                        att32[:p, g], eN[:p, g], r32[:p, g : g + 1]
                    )

                # wsum[c][g] = att[g, c] * hs[g, c] (fp16), split ACT/DVE
                wsum = wpool.tile([P, NCH, NG, NHID], dt.float16, tag="wsum")
                k = 0
                for c in range(NCH):
                    for g in range(ng):
                        if k % 32 < N_ACT_WSUM:
                            nc.scalar.mul(
                                wsum[:p, c, g, :],
                                hs_t[:p, g, c, :],
                                att32[:p, g, c : c + 1],
                            )
                        else:
                            nc.vector.tensor_scalar_mul(
                                wsum[:p, c, g, :],
                                hs_t[:p, g, c, :],
                                att32[:p, g, c : c + 1],
                            )
                        k += 1

                # channel sum: 3-level fp16 tree on DVE
                c1 = cspool.tile([P, 4, NG, NHID], dt.float16, tag="c1")
                nc.vector.tensor_add(
                    c1[:p, :, 0:ng], wsum[:p, 0:4, 0:ng], wsum[:p, 4:8, 0:ng]
                )
                c2 = cspool.tile([P, 2, NG, NHID], dt.float16, tag="c2")
                nc.vector.tensor_add(
                    c2[:p, :, 0:ng], c1[:p, 0:2, 0:ng], c1[:p, 2:4, 0:ng]
                )
                out_t = opool.tile([P, NG, NHID], dt.float16, tag="out")
                nc.vector.tensor_add(
                    out_t[:p, 0:ng], c2[:p, 0, 0:ng], c2[:p, 1, 0:ng]
                )
                nc.sync.dma_start(
                    out_d[base : base + n].rearrange("(g pp) h -> pp g h", pp=p),
                    out_t[:p, 0:ng],
                )

            for i in range(FULL_ITERS):
                body(i * NG * P, NG, P)
            if TAIL:
                body(FULL_ITERS * NG * P, 3, TAIL // 3)

    nc.compile()
    return nc


def _get_program():
    if "nc" not in _cache:
        _cache["nc"] = _build_program()
    return _cache["nc"]


def _prep_inputs(hs, meta_att_vec):
    hs = np.asarray(hs)
    v = np.asarray(meta_att_vec, dtype=np.float32)
    # vpad[h, c, j] = v[c, h] if j == c else 0  (per-channel padded lhsT)
    vpad = np.zeros((NHID, NCH, NCH), dtype=np.float16)
    v16 = v.astype(np.float16)
    for c in range(NCH):
        vpad[:, c, c] = v16[c]
    ident8 = np.eye(8, dtype=np.float32)
    hs16 = hs.astype(np.float16)
    in_maps = []
    for i in range(NCORES):
        sl = hs16[:, i * NPC : (i + 1) * NPC, :]
        in_maps.append(
            {
                "hs": np.ascontiguousarray(sl.transpose(1, 0, 2)),
                "hsT": np.ascontiguousarray(sl.transpose(0, 2, 1)),
                "vpad": vpad,
                "ident8": ident8,
            }
        )
    return in_maps


def run(hs, meta_att_vec, trace=False):
    from concourse.bass_utils import run_bass_kernel_spmd

    nc = _get_program()
    in_maps = _prep_inputs(hs, meta_att_vec)
    res = run_bass_kernel_spmd(nc, in_maps, list(range(NCORES)), trace=trace)
    out = np.concatenate(
        [res.results[i]["out"].astype(np.float32) for i in range(NCORES)], axis=0
    )
    return out, res


def kernel(hs, meta_att_vec, nnode=None):
    out, _ = run(hs, meta_att_vec, trace=False)
    return out


# revision 16
# speedup vs baseline: 1.0666x; 1.0666x over previous
"""Trainium2 Bass kernel for MetapathAggreLayer.

Computes, per node n:
    score[n, c] = sum_h hs[c, n, h] * v[c, h]        (c = 8 channels)
    att = softmax(score, axis=c)
    out[n, h]   = sum_c att[n, c] * hs[c, n, h]

Sharding: data-parallel over nodes across 8 NeuronCores (50000 nodes each).

hs is staged twice in fp16 (same total bytes as fp32 once): an [n, c, h]
interleave for the attention-weighted sum, and a [c, h, n] transpose so the
score dot-products run on the TensorEngine (contraction over h = partitions).
Output returns fp16, cast to fp32 on the host.

Per 512-node macro-tile:
  PE:  score via 8 matmuls (v_c column x hsT panel -> one PSUM row each),
       then 4 transposes to bring score back to node-partitions
  ACT: exp, part of the per-channel att*hs scalings, score PSUM->SBUF copy
  DVE: softmax sums/reciprocal/normalize, the other att*hs scalings, and the
       channel sum as a 3-level fp16 tree-add
  Pool: SWDGE descriptor-gen for the transposed loads (otherwise idle)
"""

import sys

if "/opt/trn_rl_repo" not in sys.path:
    sys.path.insert(0, "/opt/trn_rl_repo")

import numpy as np

NCH = 8
NNODE = 400000
NHID = 128
NCORES = 8
NPC = NNODE // NCORES  # 50000
P = 128
NG = 4
FULL_ITERS = NPC // (NG * P)  # 97
TAIL = NPC - FULL_ITERS * NG * P  # 336 = 3 * 112

N_ACT_WSUM = 11  # of 32 att*hs scalings, how many go to ACT (rest DVE)

_cache = {}


def _build_program():
    import concourse.bacc as bacc
    import concourse.tile as tile
    import concourse.mybir as mybir

    dt = mybir.dt
    AX = mybir.AxisListType
    AF = mybir.ActivationFunctionType

    nc = bacc.Bacc("TRN2", target_bir_lowering=False, debug=False)
    hs_d = nc.dram_tensor("hs", [NPC, NCH, NHID], dt.float16, kind="ExternalInput").ap()
    hsT_d = nc.dram_tensor(
        "hsT", [NCH, NHID, NPC], dt.float16, kind="ExternalInput"
    ).ap()
    vc_d = nc.dram_tensor("vpad", [NHID, NCH, NCH], dt.float16, kind="ExternalInput").ap()
    i32_d = nc.dram_tensor("ident8", [8, 8], dt.float32, kind="ExternalInput").ap()
    out_d = nc.dram_tensor("out", [NPC, NHID], dt.float16, kind="ExternalOutput").ap()

    with tile.TileContext(nc) as tc:
        with (
            tc.tile_pool(name="const", bufs=1) as cpool,
            tc.tile_pool(name="hs", bufs=3) as hpool,
            tc.tile_pool(name="hsT", bufs=4) as thpool,
            tc.tile_pool(name="wsum", bufs=2) as wpool,
            tc.tile_pool(name="csum", bufs=2) as cspool,
            tc.tile_pool(name="small", bufs=6) as spool,
            tc.tile_pool(name="outp", bufs=4) as opool,
            tc.tile_pool(name="ps", bufs=2, space="PSUM") as pspool,
        ):
            vpad = cpool.tile([NHID, NCH, NCH], dt.float16)
            nc.sync.dma_start(vpad[:], vc_d[:])
            I8 = cpool.tile([8, 8], dt.float32)
            nc.sync.dma_start(I8[:], i32_d[:])
            bconst = cpool.tile([P, 1], dt.float32)
            nc.vector.memset(bconst[:], -5.545177444479562)

            def body(base, ng, p):
                n = ng * p
                hs_t = hpool.tile([P, NG, NCH, NHID], dt.float16, tag="hs")
                for g in range(ng):
                    eng = nc.scalar if g == 3 else nc.sync
                    eng.dma_start(
                        hs_t[:p, g],
                        hs_d[base + g * p : base + (g + 1) * p],
                    )
                hsT_t = thpool.tile([NHID, NCH, NG * P], dt.float16, tag="hsT")
                for q in range(4):
                    nc.gpsimd.dma_start(
                        hsT_t[:, 2 * q : 2 * q + 2, 0:n],
                        hsT_d[2 * q : 2 * q + 2, :, base : base + n].rearrange(
                            "c h n -> h c n"
                        ),
                    )

                # score on PE: row c of scT <- v_c . hsT_c
                scT = pspool.tile([NCH, NG * P], dt.float32, tag="scT")
                for c in range(NCH):
                    nc.tensor.matmul(
                        scT[0:NCH, 0:n],
                        vpad[:, c, :],
                        hsT_t[:, c, 0:n],
                        start=(c == 0),
                        stop=(c == NCH - 1),
                    )
                scTs = spool.tile([NCH, NG * P], dt.float32, tag="scTs")
                nc.scalar.copy(scTs[:, 0:n], scT[:, 0:n])

                # back to node partitions: scn[p, g, c]
                scn = pspool.tile([P, NG, NCH], dt.float32, tag="scn")
                for g in range(ng):
                    nc.tensor.transpose(
                        scn[:p, g], scTs[:, g * p : (g + 1) * p], I8[:]
                    )

                # softmax over c; biased exp keeps e in range, bias cancels in r
                e32 = spool.tile([P, NG, NCH], dt.float32, tag="e32")
                nc.scalar.activation(
                    e32[:p, 0:ng], scn[:p, 0:ng], AF.Exp, bias=bconst[:p]
                )
                s32 = spool.tile([P, NG], dt.float32, tag="s32")
                nc.vector.reduce_sum(s32[:p, 0:ng], e32[:p, 0:ng], axis=AX.X)
                r32 = spool.tile([P, NG], dt.float32, tag="r32")
                nc.vector.reciprocal(r32[:p, 0:ng], s32[:p, 0:ng])
                att32 = spool.tile([P, NG, NCH], dt.float32, tag="att32")
                for g in range(ng):
                    nc.vector.tensor_scalar_mul(
                        att32[:p, g], e32[:p, g], r32[:p, g : g + 1]
                    )

                # wsum[c][g] = att[g, c] * hs[g, c] (fp16), split ACT/DVE
                wsum = wpool.tile([P, NCH, NG, NHID], dt.float16, tag="wsum")
                k = 0
                for c in range(NCH):
                    for g in range(ng):
                        if k % 32 < N_ACT_WSUM:
                            nc.scalar.mul(
                                wsum[:p, c, g, :],
                                hs_t[:p, g, c, :],
                                att32[:p, g, c : c + 1],
                            )
                        else:
                            nc.vector.tensor_scalar_mul(
                                wsum[:p, c, g, :],
                                hs_t[:p, g, c, :],
                                att32[:p, g, c : c + 1],
                            )
                        k += 1

                # channel sum: 3-level fp16 tree on DVE
                c1 = cspool.tile([P, 4, NG, NHID], dt.float16, tag="c1")
                nc.vector.tensor_add(
                    c1[:p, :, 0:ng], wsum[:p, 0:4, 0:ng], wsum[:p, 4:8, 0:ng]
                )
                c2 = cspool.tile([P, 2, NG, NHID], dt.float16, tag="c2")
                nc.vector.tensor_add(
                    c2[:p, :, 0:ng], c1[:p, 0:2, 0:ng], c1[:p, 2:4, 0:ng]
                )
                out_t = opool.tile([P, NG, NHID], dt.float16, tag="out")
                nc.vector.tensor_add(
                    out_t[:p, 0:ng], c2[:p, 0, 0:ng], c2[:p, 1, 0:ng]
                )
                nc.sync.dma_start(
                    out_d[base : base + n].rearrange("(g pp) h -> pp g h", pp=p),
                    out_t[:p, 0:ng],
                )

            for i in range(FULL_ITERS):
                body(i * NG * P, NG, P)
            if TAIL:
                body(FULL_ITERS * NG * P, 3, TAIL // 3)

    nc.compile()
    return nc


def _get_program():
    if "nc" not in _cache:
        _cache["nc"] = _build_program()
    return _cache["nc"]


def _prep_inputs(hs, meta_att_vec):
    hs = np.asarray(hs)
    v = np.asarray(meta_att_vec, dtype=np.float32)
    # vpad[h, c, j] = v[c, h] if j == c else 0  (per-channel padded lhsT)
    vpad = np.zeros((NHID, NCH, NCH), dtype=np.float16)
    v16 = v.astype(np.float16)
    for c in range(NCH):
        vpad[:, c, c] = v16[c]
    ident8 = np.eye(8, dtype=np.float32)
    hs16 = hs.astype(np.float16)
    in_maps = []
    for i in range(NCORES):
        sl = hs16[:, i * NPC : (i + 1) * NPC, :]
        in_maps.append(
            {
                "hs": np.ascontiguousarray(sl.transpose(1, 0, 2)),
                "hsT": np.ascontiguousarray(sl.transpose(0, 2, 1)),
                "vpad": vpad,
                "ident8": ident8,
            }
        )
    return in_maps


def run(hs, meta_att_vec, trace=False):
    from concourse.bass_utils import run_bass_kernel_spmd

    nc = _get_program()
    in_maps = _prep_inputs(hs, meta_att_vec)
    res = run_bass_kernel_spmd(nc, in_maps, list(range(NCORES)), trace=trace)
    out = np.concatenate(
        [res.results[i]["out"].astype(np.float32) for i in range(NCORES)], axis=0
    )
    return out, res


def kernel(hs, meta_att_vec, nnode=None):
    out, _ = run(hs, meta_att_vec, trace=False)
    return out
